# revision 15
# baseline (speedup 1.0000x reference)
"""HSN layer (2-level GNN message passing) on 8 Trainium2 NeuronCores.

out = sigmoid(A@((sigmoid(A@(x@W1_00)))@W2_00) + B@((sigmoid(B^T@(x@W1_01)))@W2_10))

Reformulated with W applied post-aggregation (A@(x@W) == (A@x)@W):
  z1 = sigmoid((A @ x) @ W1_00)        [N, C]   rows sharded over 8 cores
  z2 = sigmoid((B^T @ x) @ W1_01)      [E, C]   rows sharded
  out = sigmoid((A @ z1) @ W2_00 + (B @ z2) @ W2_10)

Per core (SPMD, one program):
  - bulk row gathers via gpsimd dma_gather (int16 bank-local indices, 2048
    idx/call, single_packet=False, 4 SWDGE queues)
  - segment-sum per 128-nnz chunk: PE matmul psum += S^T @ G with the one-hot
    S[q,m] = val_q * (rowlocal_q == m) built in ONE fused DVE tensor_scalar
    (iota is_equal rowlocal, then mult val)
  - per-tile epilogue: PSUM -> bf16 -> PE transpose -> @W -> sigmoid
    (level 2 uses a hi/lo bf16 split of both the sums and W for f32-like
     precision through the PE)
  - AllGather z1/z2 between levels, overlapped with independent compute
"""
import os
import sys
import types
import numpy as np
import ml_dtypes

# ---- inline axon NTFF profile hook shim (antenv.axon_hooks missing in image) ----
def _install_axon_shim():
    try:
        import antenv
    except ImportError:
        return
    if "antenv.axon_hooks" in sys.modules:
        return
    mod = types.ModuleType("antenv.axon_hooks")
    mod._hook = None
    mod.set_axon_ntff_profile_hook = lambda h: setattr(mod, "_hook", h)
    mod.get_axon_ntff_profile_hook = lambda: mod._hook
    sys.modules["antenv.axon_hooks"] = mod
    antenv.axon_hooks = mod
    try:
        from trn_agent_boot.trn_boot import _ntff_profile_via_ctypes
        hook = _ntff_profile_via_ctypes("/opt/axon/libaxon_pjrt.so")
        if hook is not None:
            mod._hook = hook
    except Exception:
        pass


_install_axon_shim()

import concourse.bacc as bacc
import concourse.bass as bass
import concourse.mybir as mybir
import concourse.tile as tile
from concourse.tile_rust import add_dep_helper
from concourse.bass_utils import run_bass_kernel_spmd
from concourse.library_config import mlp

bf16 = mybir.dt.bfloat16
f32 = mybir.dt.float32
i16 = mybir.dt.int16
nbf = ml_dtypes.bfloat16

N_NODES = 100000
N_EDGES = 200000
C = 128
NCORES = 8
BANK = 32768
NSH = 12544   # node rows per core (98 tiles)
ESH = 25088   # edge rows per core (196 tiles)
NP_ = NSH * NCORES
EP_ = ESH * NCORES
CALL_BLOCKS = 16
NIDX_CALL = CALL_BLOCKS * 128
IDXGRP = 4    # gather calls per idx-load DMA

last_exec_time_ns = None
last_results = None


# ---------------------------------------------------------------- host prep
class Spmm:
    """Chunked-CSR structure for one SpMM, shared across cores (max-padded)."""

    def __init__(self, rows, cols, vals, shard, n_tab, name, ship_S=False):
        self.name = name
        self.shard = shard
        self.ntiles = shard // 128
        self.nbanks = (n_tab + BANK - 1) // BANK
        NT, NB = self.ntiles, self.nbanks

        core = rows // shard
        lrow = rows % shard
        tile_ = lrow >> 7
        rloc = (lrow & 127).astype(np.int32)
        bank = cols >> 15
        bidx = (cols & 32767).astype(np.int32)

        cell = (core.astype(np.int64) * NT + tile_) * NB + bank
        order = np.argsort(cell, kind="stable")
        cell_s = cell[order]
        bidx_s = bidx[order]
        vals_s = vals[order]
        rloc_s = rloc[order]

        ncells = NCORES * NT * NB
        counts = np.bincount(cell_s, minlength=ncells).astype(np.int64)
        counts_c = counts.reshape(NCORES, NT, NB)
        nch_tb = (counts_c.max(axis=0) + 127) // 128   # [NT, NB]
        empty = nch_tb.sum(axis=1) == 0
        nch_tb[empty, 0] = 1
        self.nch_tile = nch_tb.sum(axis=1)
        self.totchunks = int(self.nch_tile.sum())

        slots_tb = nch_tb * 128
        cellbase = np.zeros(NT * NB, dtype=np.int64)
        np.cumsum(slots_tb.reshape(-1)[:-1], out=cellbase[1:])
        self.totslots = int(slots_tb.sum())

        idx_slots = np.zeros((NCORES, self.totslots), dtype=np.int16)
        val_slots = np.zeros((NCORES, self.totslots), dtype=np.float32)
        rl_slots = np.zeros((NCORES, self.totslots), dtype=np.int32)

        seg_start = np.zeros(ncells + 1, dtype=np.int64)
        np.cumsum(counts, out=seg_start[1:])
        rank = np.arange(len(cell_s), dtype=np.int64) - seg_start[cell_s]
        core_s = cell_s // (NT * NB)
        tb_s = cell_s % (NT * NB)
        pos = cellbase[tb_s] + rank
        for k in range(NCORES):
            m = core_s == k
            idx_slots[k, pos[m]] = bidx_s[m].astype(np.int16)
            val_slots[k, pos[m]] = vals_s[m]
            rl_slots[k, pos[m]] = rloc_s[m]

        # pack chunks into per-bank calls, tile-major
        open_call = {}
        calls = []
        chunk_call = []
        tile_chunks = []
        gch = 0
        for t in range(NT):
            tl = []
            for b in range(NB):
                base = cellbase[t * NB + b]
                for c in range(int(nch_tb[t, b])):
                    if b not in open_call:
                        open_call[b] = len(calls)
                        calls.append({"bank": b, "blocks": []})
                    cid = open_call[b]
                    blk = len(calls[cid]["blocks"])
                    calls[cid]["blocks"].append(base + c * 128)
                    chunk_call.append((cid, blk))
                    tl.append(gch)
                    gch += 1
                    if blk + 1 == CALL_BLOCKS:
                        del open_call[b]
            tile_chunks.append(tl)
        self.calls = calls
        self.chunk_call = chunk_call
        self.tile_chunks = tile_chunks
        self.ncalls = len(calls)
        self.tile_last_call = [max(chunk_call[g][0] for g in tl) for tl in tile_chunks]

        # idx data partition-major: [NCORES, 128, ncalls*128] int16
        self.idx_pm = np.zeros((NCORES, 128, self.ncalls * 128), dtype=np.int16)
        for cid, call in enumerate(calls):
            flat = np.zeros((NCORES, NIDX_CALL), dtype=np.int16)
            for j, sb in enumerate(call["blocks"]):
                flat[:, j * 128:(j + 1) * 128] = idx_slots[:, sb:sb + 128]
            w = flat.reshape(NCORES, 128, 16).transpose(0, 2, 1)   # [NC,16,128]
            self.idx_pm[:, :, cid * 128:(cid + 1) * 128] = np.tile(w, (1, 8, 1))

        self.tile_coff = np.zeros(NT + 1, dtype=np.int64)
        np.cumsum(self.nch_tile, out=self.tile_coff[1:])
        self.max_nch = int(self.nch_tile.max())
        self.vrl = None
        self.s_stream = None
        if ship_S:
            # dense scaled one-hot stream: chunk ch -> [:, ch*128:(ch+1)*128]
            self.s_stream = np.zeros((NCORES, 128, self.totchunks * 128), dtype=nbf)
            slot = np.arange(self.totslots, dtype=np.int64)
            q = (slot % 128).astype(np.int64)
            col = (slot // 128) * 128 + 0  # + rl per core below
            for k in range(NCORES):
                self.s_stream[k, q, col + rl_slots[k]] = val_slots[k].astype(nbf)
        else:
            # combined val|rl per tile: [NCORES, 128, 2*totchunks] bf16
            vm = val_slots.reshape(NCORES, self.totchunks, 128).transpose(0, 2, 1)
            rm = rl_slots.reshape(NCORES, self.totchunks, 128).transpose(0, 2, 1)
            self.vrl = np.zeros((NCORES, 128, 2 * self.totchunks), dtype=nbf)
            for t in range(NT):
                o, n = int(self.tile_coff[t]), int(self.nch_tile[t])
                self.vrl[:, :, 2 * o:2 * o + n] = vm[:, :, o:o + n].astype(nbf)
                self.vrl[:, :, 2 * o + n:2 * o + 2 * n] = rm[:, :, o:o + n].astype(nbf)


class SpmmEmitter:
    """Emits gather calls + per-tile chunk matmuls for one SpMM stream.

    mode "ship": S matrices streamed from DRAM (s_dram), loaded per SLGRP
    chunks. mode "build": S built on DVE from vrl (grouped is_equal+mult).
    """

    SLGRP = 16   # chunks per shipped-S load
    SGRP = 4     # chunks per built-S DVE op pair

    def __init__(self, nc, pools, sp, tab_dram, idx_dram, aux_dram, ps_tag,
                 gtag="g", mode="build", s_load_engines=None):
        self.nc = nc
        self.pools = pools
        self.sp = sp
        self.tab = tab_dram
        self.idx = idx_dram
        self.aux = aux_dram   # vrl (build) or s_stream (ship)
        self.ps_tag = ps_tag
        self.gtag = gtag
        self.mode = mode
        self.call_tiles = [None] * sp.ncalls
        self.next_call = 0
        self.first_gather_inst = None
        self.s_tiles = {}     # sgroup index -> tile (ship mode)
        self.next_sload = 0
        self.s_load_engines = s_load_engines or [nc.sync, nc.scalar]

    def emit_calls_until(self, last_cid):
        nc, sp = self.nc, self.sp
        pools = self.pools
        while self.next_call <= last_cid:
            c0 = self.next_call
            ng = min(IDXGRP, sp.ncalls - c0)
            idx_t = pools["idxp"].tile([128, IDXGRP * 128], i16, tag="idx",
                                       name=f"idx_{sp.name}_{c0}")
            nc.sync.dma_start(idx_t[:, :ng * 128], self.idx[:, c0 * 128:(c0 + ng) * 128])
            for j in range(ng):
                cid = c0 + j
                b = sp.calls[cid]["bank"]
                lo = b * BANK
                hi = min(lo + BANK, self.tab.shape[0])
                gt = pools["gath"].tile([128, CALL_BLOCKS, C], bf16, tag=self.gtag,
                                        name=f"g_{sp.name}_{cid}",
                                        bufs=16 if self.gtag == "gB" else 10)
                inst = nc.gpsimd.dma_gather(
                    gt[:], self.tab[lo:hi, :], idx_t[:, j * 128:(j + 1) * 128],
                    NIDX_CALL, NIDX_CALL, C,
                    single_packet=False, queue_num=cid % 4,
                )
                if self.first_gather_inst is None:
                    self.first_gather_inst = inst.ins
                self.call_tiles[cid] = gt
            self.next_call = c0 + ng

    def _s_tile_for(self, gch):
        """ship mode: ensure the S group containing global chunk gch is loaded."""
        sg = gch // self.SLGRP
        if sg not in self.s_tiles:
            nc, sp = self.nc, self.sp
            c0 = sg * self.SLGRP
            n = min(self.SLGRP, sp.totchunks - c0)
            st = self.pools["sload"].tile([128, self.SLGRP * 128], bf16, tag="sload",
                                          name=f"sl_{sp.name}_{sg}")
            eng = self.s_load_engines[sg % len(self.s_load_engines)]
            eng.dma_start(st[:, :n * 128], self.aux[:, c0 * 128:(c0 + n) * 128])
            # keep only a window of recent groups referenced
            self.s_tiles[sg] = st
            for k in list(self.s_tiles):
                if k < sg - 6:
                    del self.s_tiles[k]
        return self.s_tiles[sg]

    def emit_tile_psum(self, t):
        nc, sp, pools = self.nc, self.sp, self.pools
        self.emit_calls_until(sp.tile_last_call[t])
        nch = int(sp.nch_tile[t])
        coff = int(sp.tile_coff[t])
        psum = pools["ps"].tile([128, 128], f32, tag=self.ps_tag,
                                name=f"ps_{sp.name}_{t}",
                                bufs=3 if self.ps_tag == "psS" else 2)
        if self.mode == "ship":
            for i, gch in enumerate(sp.tile_chunks[t]):
                cid, blk = sp.chunk_call[gch]
                st = self._s_tile_for(gch)
                o = (gch % self.SLGRP) * 128
                nc.tensor.matmul(
                    psum[:], lhsT=st[:, o:o + 128],
                    rhs=self.call_tiles[cid][:, blk, :],
                    start=(i == 0), stop=(i == nch - 1),
                )
            return psum
        # build mode
        vr = pools["vrp"].tile([128, 2 * sp.max_nch], bf16, tag="vr",
                               name=f"vr_{sp.name}_{t}")
        nc.scalar.dma_start(vr[:, :2 * nch], self.aux[:, 2 * coff:2 * coff + 2 * nch])
        iota = pools["iota"]
        SG = self.SGRP
        s_groups = []
        for g0 in range(0, nch, SG):
            g = min(SG, nch - g0)
            S = pools["sp"].tile([128, SG * 128], bf16, tag="S",
                                 name=f"S_{sp.name}_{t}_{g0}")
            nc.vector.tensor_tensor(
                out=S[:, :g * 128].rearrange("p (g m) -> p g m", m=128),
                in0=iota[:, :g * 128].rearrange("p (g m) -> p g m", m=128),
                in1=vr[:, nch + g0:nch + g0 + g].to_broadcast([128, g, 128]),
                op=mybir.AluOpType.is_equal,
            )
            nc.vector.tensor_tensor(
                out=S[:, :g * 128].rearrange("p (g m) -> p g m", m=128),
                in0=S[:, :g * 128].rearrange("p (g m) -> p g m", m=128),
                in1=vr[:, g0:g0 + g].to_broadcast([128, g, 128]),
                op=mybir.AluOpType.mult,
            )
            s_groups.append(S)
        for i, gch in enumerate(sp.tile_chunks[t]):
            cid, blk = sp.chunk_call[gch]
            S = s_groups[i // SG]
            nc.tensor.matmul(
                psum[:], lhsT=S[:, (i % SG) * 128:(i % SG) * 128 + 128],
                rhs=self.call_tiles[cid][:, blk, :],
                start=(i == 0), stop=(i == nch - 1),
            )
        return psum


def _build_program(spA, spB1, spB2):
    nc = bacc.Bacc("TRN2", target_bir_lowering=False, debug=False,
                   num_devices=NCORES, num_swdge_queues=4)

    x_tab = nc.dram_tensor("x_tab", [NP_, C], bf16, kind="ExternalInput")
    iota_d = nc.dram_tensor("iota", [128, 512], bf16, kind="ExternalInput")
    ident_d = nc.dram_tensor("ident", [128, 128], bf16, kind="ExternalInput")
    w_d = {}
    for wname in ("W1_00", "W1_01", "W2_00h", "W2_00l", "W2_10h", "W2_10l"):
        w_d[wname] = nc.dram_tensor(wname, [C, C], bf16, kind="ExternalInput")
    idxA_d = nc.dram_tensor("idxA", [128, spA.ncalls * 128], i16, kind="ExternalInput")
    sA_d = nc.dram_tensor("sA", [128, spA.totchunks * 128], bf16, kind="ExternalInput")
    idxB1_d = nc.dram_tensor("idxB1", [128, spB1.ncalls * 128], i16, kind="ExternalInput")
    vrlB1_d = nc.dram_tensor("vrlB1", [128, 2 * spB1.totchunks], bf16, kind="ExternalInput")
    idxB2_d = nc.dram_tensor("idxB2", [128, spB2.ncalls * 128], i16, kind="ExternalInput")
    vrlB2_d = nc.dram_tensor("vrlB2", [128, 2 * spB2.totchunks], bf16, kind="ExternalInput")

    out_d = nc.dram_tensor("out", [NSH, C], f32, kind="ExternalOutput")

    z1_loc = nc.dram_tensor("z1_loc", [NSH, C], bf16)
    z2_loc = nc.dram_tensor("z2_loc", [ESH, C], bf16)
    z1_tab = nc.dram_tensor("z1_tab", [NP_, C], bf16)
    z2_tab = nc.dram_tensor("z2_tab", [EP_, C], bf16)
    l2b_stage = nc.dram_tensor("l2b_stage", [NSH, C], f32)

    with tile.TileContext(nc) as tc:
        with (
            tc.tile_pool(name="gath", bufs=10) as gath,
            tc.tile_pool(name="idxp", bufs=4) as idxp,
            tc.tile_pool(name="vrp", bufs=6) as vrp,
            tc.tile_pool(name="spool", bufs=10) as spool,
            tc.tile_pool(name="sload", bufs=8) as sloadp,
            tc.tile_pool(name="cpool", bufs=4) as cpool,
            tc.tile_pool(name="zpool", bufs=4) as zpool,
            tc.tile_pool(name="consts", bufs=1) as consts,
            tc.tile_pool(name="psum", bufs=2, space="PSUM") as pspool,
        ):
            nc.gpsimd.load_library(mlp)
            tc.strict_bb_all_engine_barrier()

            iota_t = consts.tile([128, 512], bf16, tag="iota")
            ident_t = consts.tile([128, 128], bf16, tag="ident")
            nc.sync.dma_start(iota_t[:], iota_d[:])
            nc.sync.dma_start(ident_t[:], ident_d[:])
            w_t = {}
            for wname in w_d:
                w_t[wname] = consts.tile([C, C], bf16, tag=wname, name=f"w_{wname}")
                nc.sync.dma_start(w_t[wname][:], w_d[wname][:])
            tc.strict_bb_all_engine_barrier()

            pools = {
                "gath": gath, "idxp": idxp, "vrp": vrp, "sp": spool,
                "sload": sloadp, "ps": pspool, "iota": iota_t[:],
            }

            def l1_tile_epilogue(t, psum, w_tile, z_dram):
                sbS = cpool.tile([128, 128], bf16, tag="sbS", name=f"sbS_{z_dram.name}_{t}")
                nc.vector.tensor_copy(sbS[:], psum[:])
                psT = pspool.tile([128, 128], bf16, tag="psT", name=f"psT_{z_dram.name}_{t}", bufs=1)
                nc.tensor.transpose(psT[:], sbS[:], ident_t[:])
                sbT = cpool.tile([128, 128], bf16, tag="sbT", name=f"sbT_{z_dram.name}_{t}")
                nc.vector.tensor_copy(sbT[:], psT[:])
                psZ = pspool.tile([128, 128], f32, tag="psZ", name=f"psZ_{z_dram.name}_{t}")
                nc.tensor.matmul(psZ[:], lhsT=sbT[:], rhs=w_tile[:], start=True, stop=True)
                zt = zpool.tile([128, 128], bf16, tag="zt", name=f"zt_{z_dram.name}_{t}")
                nc.scalar.activation(zt[:], psZ[:], mybir.ActivationFunctionType.Sigmoid)
                return nc.scalar.dma_start(z_dram[t * 128:t * 128 + 128, :], zt[:])

            # ---------------- L1-B (first: its AllGather is the big one) ----
            z2_writes = []
            with nc.named_scope("L1B"):
                emB1 = SpmmEmitter(nc, pools, spB1, x_tab, idxB1_d, vrlB1_d, "psS", mode="build")
                for t in range(spB1.ntiles):
                    ps = emB1.emit_tile_psum(t)
                    z2_writes.append(l1_tile_epilogue(t, ps, w_t["W1_01"], z2_loc))
            ag2 = nc.gpsimd.collective_compute(
                "AllGather", mybir.AluOpType.bypass,
                replica_groups=[list(range(NCORES))],
                ins=[z2_loc.ap().opt()], outs=[z2_tab.ap().opt()],
            )

            # ---------------- L1-A ----------------
            for w in z2_writes:
                add_dep_helper(ag2.ins, w.ins, True, "AG2 reads z2_loc")

            def hi_lo(t, psum, side):
                sb_hi = cpool.tile([128, 128], bf16, tag=f"hi{side}", name=f"hi{side}_{t}")
                nc.vector.tensor_copy(sb_hi[:], psum[:])
                sb_lo = cpool.tile([128, 128], bf16, tag=f"lo{side}", name=f"lo{side}_{t}")
                nc.vector.tensor_tensor(out=sb_lo[:], in0=psum[:], in1=sb_hi[:],
                                        op=mybir.AluOpType.subtract)
                outs = []
                for nm, sb in (("h", sb_hi), ("l", sb_lo)):
                    psT = pspool.tile([128, 128], bf16, tag="psT",
                                      name=f"psT{side}{nm}_{t}", bufs=1)
                    nc.tensor.transpose(psT[:], sb[:], ident_t[:])
                    sbT = cpool.tile([128, 128], bf16, tag=f"sbT{side}{nm}",
                                     name=f"sbT{side}{nm}_{t}")
                    nc.vector.tensor_copy(sbT[:], psT[:])
                    outs.append(sbT)
                return outs

            # ---------------- L2-B early (overlaps L1A): stage (B@z2)@W2_10 ----
            b2_writes = []
            with nc.named_scope("L2B"):
                emB2 = SpmmEmitter(nc, pools, spB2, z2_tab, idxB2_d, vrlB2_d,
                                   "psSB", gtag="gB", mode="build")
                for t in range(spB2.ntiles):
                    psB = emB2.emit_tile_psum(t)
                    if t == 0:
                        add_dep_helper(emB2.first_gather_inst, ag2.ins, True,
                                       "L2B gathers read z2_tab from AllGather2")
                    tBh, tBl = hi_lo(t, psB, "B")
                    psZB = pspool.tile([128, 128], f32, tag="psZ", name=f"psZB_{t}")
                    nc.tensor.matmul(psZB[:], lhsT=tBh[:], rhs=w_t["W2_10h"][:], start=True, stop=False)
                    nc.tensor.matmul(psZB[:], lhsT=tBh[:], rhs=w_t["W2_10l"][:], start=False, stop=False)
                    nc.tensor.matmul(psZB[:], lhsT=tBl[:], rhs=w_t["W2_10h"][:], start=False, stop=True)
                    stb = zpool.tile([128, 128], f32, tag="stb", name=f"stb_{t}")
                    nc.vector.tensor_copy(stb[:], psZB[:])
                    b2_writes.append(nc.scalar.dma_start(l2b_stage[t * 128:t * 128 + 128, :], stb[:]))

            z1_writes = []
            with nc.named_scope("L1A"):
                emA = SpmmEmitter(nc, pools, spA, x_tab, idxA_d, sA_d, "psS", mode="ship")
                for t in range(spA.ntiles):
                    ps = emA.emit_tile_psum(t)
                    z1_writes.append(l1_tile_epilogue(t, ps, w_t["W1_00"], z1_loc))
            ag1 = nc.gpsimd.collective_compute(
                "AllGather", mybir.AluOpType.bypass,
                replica_groups=[list(range(NCORES))],
                ins=[z1_loc.ap().opt()], outs=[z1_tab.ap().opt()],
            )
            for w in z1_writes:
                add_dep_helper(ag1.ins, w.ins, True, "AG1 reads z1_loc")

            # ---------------- L2-A + merge ----------------
            with nc.named_scope("L2A"):
                emA2 = SpmmEmitter(nc, pools, spA, z1_tab, idxA_d, sA_d, "psS", mode="ship")
                for t in range(spA.ntiles):
                    psA = emA2.emit_tile_psum(t)
                    if t == 0:
                        add_dep_helper(emA2.first_gather_inst, ag1.ins, True,
                                       "L2A gathers read z1_tab from AllGather1")
                    tAh, tAl = hi_lo(t, psA, "A")
                    psZ = pspool.tile([128, 128], f32, tag="psZ", name=f"psZ2_{t}")
                    nc.tensor.matmul(psZ[:], lhsT=tAh[:], rhs=w_t["W2_00h"][:], start=True, stop=False)
                    nc.tensor.matmul(psZ[:], lhsT=tAh[:], rhs=w_t["W2_00l"][:], start=False, stop=False)
                    nc.tensor.matmul(psZ[:], lhsT=tAl[:], rhs=w_t["W2_00h"][:], start=False, stop=True)
                    bt = zpool.tile([128, 128], f32, tag="bt", name=f"bt_{t}")
                    rd = nc.sync.dma_start(bt[:], l2b_stage[t * 128:t * 128 + 128, :])
                    add_dep_helper(rd.ins, b2_writes[t].ins, True, "stage RAW")
                    st = zpool.tile([128, 128], f32, tag="st2", name=f"st2_{t}")
                    nc.vector.tensor_add(out=st[:], in0=bt[:], in1=psZ[:])
                    ot = zpool.tile([128, 128], f32, tag="ot", name=f"ot_{t}")
                    nc.scalar.activation(ot[:], st[:], mybir.ActivationFunctionType.Sigmoid)
                    nc.scalar.dma_start(out_d[t * 128:t * 128 + 128, :], ot[:])

    nc.compile()
    return nc


def kernel(x, W1_00, W1_01, W2_00, W2_10, adj_rows, adj_cols, adj_vals,
           inc_rows, inc_cols, inc_vals):
    global last_exec_time_ns, last_results
    adj_rows = np.asarray(adj_rows, np.int64)
    adj_cols = np.asarray(adj_cols, np.int64)
    inc_rows = np.asarray(inc_rows, np.int64)
    inc_cols = np.asarray(inc_cols, np.int64)

    spA = Spmm(adj_rows, adj_cols, np.asarray(adj_vals, np.float32), NSH, NP_, "A", ship_S=True)
    spB1 = Spmm(inc_cols, inc_rows, np.asarray(inc_vals, np.float32), ESH, NP_, "B1")
    spB2 = Spmm(inc_rows, inc_cols, np.asarray(inc_vals, np.float32), NSH, EP_, "B2")

    nc = _build_program(spA, spB1, spB2)

    x_pad = np.zeros((NP_, C), dtype=nbf)
    x_pad[:N_NODES] = np.asarray(x, np.float32).astype(nbf)
    iota_np = np.tile(np.arange(128, dtype=np.float32).astype(nbf)[None, :], (128, 4))
    ident_np = np.eye(128, dtype=np.float32).astype(nbf)

    def hl(w):
        w = np.asarray(w, np.float32)
        hi = w.astype(nbf)
        lo = (w - hi.astype(np.float32)).astype(nbf)
        return hi, lo

    w200h, w200l = hl(W2_00)
    w210h, w210l = hl(W2_10)

    in_maps = []
    for k in range(NCORES):
        in_maps.append({
            "x_tab": x_pad,
            "iota": iota_np,
            "ident": ident_np,
            "W1_00": np.asarray(W1_00, np.float32).astype(nbf),
            "W1_01": np.asarray(W1_01, np.float32).astype(nbf),
            "W2_00h": w200h, "W2_00l": w200l,
            "W2_10h": w210h, "W2_10l": w210l,
            "idxA": spA.idx_pm[k],
            "sA": spA.s_stream[k],
            "idxB1": spB1.idx_pm[k],
            "vrlB1": spB1.vrl[k],
            "idxB2": spB2.idx_pm[k],
            "vrlB2": spB2.vrl[k],
        })

    trace = os.environ.get("BASS_KERNEL_TRACE", "0") == "1"
    res = run_bass_kernel_spmd(nc, in_maps, core_ids=list(range(NCORES)), trace=trace)
    last_exec_time_ns = res.exec_time_ns
    last_results = res

    out = np.zeros((N_NODES, C), dtype=np.float32)
    for k in range(NCORES):
        lo = k * NSH
        hi = min(lo + NSH, N_NODES)
        if hi > lo:
            out[lo:hi] = res.results[k]["out"][:hi - lo]
    return out
